# revision 1
# baseline (speedup 1.0000x reference)
"""Self-contained Trainium2 kernel for nn_Attention_56607668961538.

kernel(**inputs) takes the FULL unsharded inputs (B=16, N=1024, C=1024),
shards data-parallel over batch across 8 NeuronCores, runs a Bass/Tile
attention kernel per core via run_bass_kernel_spmd, and gathers the full
output.  See build_attention below for the on-device layout strategy.
"""

import sys

sys.path.insert(0, "/opt/trn_rl_repo")

import numpy as np

from contextlib import ExitStack

import numpy as np

import concourse.bass as bass
import concourse.mybir as mybir
import concourse.tile as tile

F32 = mybir.dt.float32
F32R = mybir.dt.float32r
EPS = 1e-6


def build_attention(nc, B_local, N, C, H, reps=1):
    Dh = C // H
    assert Dh == 64
    KT = C // 128           # contraction k-tiles
    NT = N // 128           # token m-tiles per batch
    FQK = 2 * C // 128      # q+k feature tiles
    TCH = min(512, N)       # token chunk for phase 1
    NCH = N // TCH
    VW = min(256, C)        # v-weight chunk width
    VH = VW // Dh           # heads per v chunk
    scale = Dh ** -0.5
    E = 65                  # Dh + ones column

    def halves():
        return [(off, min(512, N - off)) for off in range(0, N, 512)]

    # ---- external I/O ----
    xT = nc.dram_tensor("xT", [B_local, C, N], F32R, kind="ExternalInput").ap()
    qk_wT = nc.dram_tensor("qk_wT", [2 * C // 128, 128, C], F32R,
                           kind="ExternalInput").ap()
    v_wT = nc.dram_tensor("v_wT", [C, C], F32R, kind="ExternalInput").ap()
    proj_wT = nc.dram_tensor("proj_wT", [C // 128, 128, C], F32R,
                             kind="ExternalInput").ap()
    proj_b = nc.dram_tensor("proj_b", [C], F32, kind="ExternalInput").ap()
    mask_ssq = nc.dram_tensor("mask_ssq", [C, H], F32R, kind="ExternalInput").ap()
    w_sel = nc.dram_tensor("w_sel", [H, C], F32R, kind="ExternalInput").ap()
    sel_q = nc.dram_tensor("sel_q", [H, C], F32R, kind="ExternalInput").ap()
    den_sel = nc.dram_tensor("den_sel", [H, C], F32R, kind="ExternalInput").ap()
    vinit = nc.dram_tensor("vinit", [128, H], F32R, kind="ExternalInput").ap()
    yT = nc.dram_tensor("yT", [B_local, C, N], F32, kind="ExternalOutput").ap()

    # ---- internal DRAM ----
    qkT_d = nc.dram_tensor("qkT_d", [B_local, 2 * C, N], F32R, kind="Internal").ap()
    attn_d = nc.dram_tensor("attn_d", [B_local, C, N], F32R, kind="Internal").ap()
    ik_d = nc.dram_tensor("ik_d", [B_local, H, N], F32, kind="Internal").ap()

    with tile.TileContext(nc) as tc, ExitStack() as ctx:
        singles = ctx.enter_context(tc.tile_pool(name="singles", bufs=1))
        xp = ctx.enter_context(tc.tile_pool(name="xp", bufs=2))
        wp = ctx.enter_context(tc.tile_pool(name="wp", bufs=2))
        vwp = ctx.enter_context(tc.tile_pool(name="vwp", bufs=1))
        stagep = ctx.enter_context(tc.tile_pool(name="stagep", bufs=2))
        sqp = ctx.enter_context(tc.tile_pool(name="sqp", bufs=1))
        vainp = ctx.enter_context(tc.tile_pool(name="vainp", bufs=NT + 1))
        statp = ctx.enter_context(tc.tile_pool(name="statp", bufs=1))
        pairp = ctx.enter_context(tc.tile_pool(name="pairp", bufs=2))
        ptp = ctx.enter_context(tc.tile_pool(name="ptp", bufs=2))
        aop = ctx.enter_context(tc.tile_pool(name="aop", bufs=1))
        atnp = ctx.enter_context(tc.tile_pool(name="atnp", bufs=KT))
        pwp = ctx.enter_context(tc.tile_pool(name="pwp", bufs=2))
        ystp = ctx.enter_context(tc.tile_pool(name="ystp", bufs=2))

        mmps = ctx.enter_context(tc.tile_pool(name="mmps", bufs=2, space="PSUM"))
        stps = ctx.enter_context(tc.tile_pool(name="stps", bufs=2, space="PSUM"))
        avps = ctx.enter_context(tc.tile_pool(name="avps", bufs=1, space="PSUM"))

        # ---- constants ----
        mask_sb = singles.tile([128, KT, H], F32R)
        nc.sync.dma_start(out=mask_sb, in_=mask_ssq.rearrange("(k p) h -> p k h", p=128))
        wsel_sb = singles.tile([H, C], F32R)
        nc.sync.dma_start(out=wsel_sb, in_=w_sel)
        selq_sb = singles.tile([H, C], F32R)
        nc.sync.dma_start(out=selq_sb, in_=sel_q)
        densel_sb = singles.tile([H, C], F32R)
        nc.sync.dma_start(out=densel_sb, in_=den_sel)
        bias_sb = singles.tile([128, KT], F32)
        nc.sync.dma_start(out=bias_sb, in_=proj_b.rearrange("(k p) -> p k", p=128))
        eps_sb = singles.tile([H, 1], F32)
        nc.vector.memset(eps_sb, EPS)

        loop = ctx.enter_context(tc.For_i(0, reps, 1)) if reps > 1 else None
        for b in range(B_local):
            # ================= phase 1: qkv projection + ssq stats ==========
            ssq_q = statp.tile([H, N], F32, tag="ssqq")
            ssq_k = statp.tile([H, N], F32, tag="ssqk")
            x_sb = {}
            for tcn in range(NCH):
                xt = xp.tile([128, KT, TCH], F32R, tag="x")
                nc.gpsimd.dma_start(
                    out=xt,
                    in_=xT[b, :, tcn * TCH:(tcn + 1) * TCH].rearrange(
                        "(k p) t -> p k t", p=128))
                x_sb[tcn] = xt
                tsl = slice(tcn * TCH, (tcn + 1) * TCH)
                for ft in range(FQK):
                    wt = wp.tile([128, KT, 128], F32R, tag="qkw")
                    nc.gpsimd.dma_start(
                        out=wt, in_=qk_wT[ft].rearrange("p (k f) -> p k f", f=128))
                    ps = mmps.tile([128, TCH], F32, tag="mm")
                    for k in range(KT):
                        nc.tensor.matmul(ps, wt[:, k], xt[:, k],
                                         start=(k == 0), stop=(k == KT - 1))
                    st = stagep.tile([128, TCH], F32R, tag="stage")
                    nc.vector.tensor_copy(st, ps)
                    nc.sync.dma_start(out=qkT_d[b, ft * 128:(ft + 1) * 128, tsl],
                                      in_=st)
                    sq = sqp.tile([128, TCH], F32R, tag="sq")
                    nc.vector.tensor_mul(sq, st, st)
                    ps2 = mmps.tile([128, TCH], F32, tag="mm")
                    nc.tensor.matmul(ps2[:H], mask_sb[:, ft % KT], sq,
                                     start=True, stop=True)
                    acc = ssq_q if ft < KT else ssq_k
                    if ft % KT == 0:
                        nc.vector.tensor_copy(acc[:, tsl], ps2[:H])
                    else:
                        nc.vector.tensor_add(acc[:, tsl], acc[:, tsl], ps2[:H])

            # V part: token-major into persistent augmented SBUF tiles
            va_t = {}
            for vc in range(C // VW):
                vwt = vwp.tile([128, KT, VW], F32R, tag="vw")
                nc.gpsimd.dma_start(
                    out=vwt,
                    in_=v_wT[:, vc * VW:(vc + 1) * VW].rearrange(
                        "(k p) f -> p k f", p=128))
                for tcn in range(NCH):
                    for tm in range(TCH // 128):
                        j = tcn * (TCH // 128) + tm
                        if vc == 0:
                            va_t[j] = vainp.tile([128, H, E], F32R, tag="vain",
                                                 name=f"vain_{b}_{j}")
                            nc.sync.dma_start(out=va_t[j][:, :, 64:65],
                                              in_=vinit.unsqueeze(-1))
                        ps = mmps.tile([128, VW], F32, tag="mm")
                        for k in range(KT):
                            nc.tensor.matmul(
                                ps, x_sb[tcn][:, k, tm * 128:(tm + 1) * 128],
                                vwt[:, k], start=(k == 0), stop=(k == KT - 1))
                        nc.vector.tensor_copy(
                            va_t[j][:, vc * VH:(vc + 1) * VH, 0:64],
                            ps.rearrange("p (h e) -> p h e", e=64))

            # ================= rmsnorm stats tail ===========================
            invr = {}
            for nm, acc in (("q", ssq_q), ("k", ssq_k)):
                ivr = statp.tile([H, N], F32R, tag="invr" + nm)
                for off, w in halves():
                    rms = statp.tile([H, TCH], F32, tag="rms")
                    nc.scalar.activation(out=rms[:, :w], in_=acc[:, off:off + w],
                                         func=mybir.ActivationFunctionType.Sqrt,
                                         bias=eps_sb, scale=1.0 / Dh)
                    inv = statp.tile([H, TCH], F32, tag="inv")
                    nc.vector.reciprocal_approx_fast(out=inv[:, :w],
                                                     in_=rms[:, :w])
                    nc.vector.tensor_copy(ivr[:, off:off + w], inv[:, :w])
                invr[nm] = ivr

            # token-major invr_k for the exp per-partition scale:
            # ikT[p, j, h] = invr_k[h, j*128 + p]
            ikT = statp.tile([128, NT, H], F32, tag="ikT")
            nc.sync.dma_start(out=ik_d[b], in_=invr["k"].bitcast(F32))
            for h in range(H):
                nc.sync.dma_start(
                    out=ikT[:, :, h],
                    in_=ik_d[b, h].rearrange("(j q) -> q j", q=128))

            # ================= attention ====================================
            den_all = statp.tile([H, N], F32, tag="den")
            for h in range(H):
                qt = pairp.tile([64, N], F32R, tag="qt")
                nc.sync.dma_start(out=qt, in_=qkT_d[b, h * 64:(h + 1) * 64, :])
                kt = pairp.tile([64, N], F32R, tag="kt")
                nc.sync.dma_start(out=kt,
                                  in_=qkT_d[b, C + h * 64:C + (h + 1) * 64, :])
                for off, w in halves():
                    sl = slice(off, off + w)
                    bq = mmps.tile([128, TCH], F32, tag="mm")
                    nc.tensor.matmul(bq[:64, :w], selq_sb[:, h * 64:(h + 1) * 64],
                                     invr["q"][:, sl], start=True, stop=True)
                    nc.vector.tensor_mul(qt[:, sl], qt[:, sl], bq[:64, :w])

                av = avps.tile([128, N], F32, tag="av")
                for j in range(NT):
                    va = va_t[j][:, h, :]
                    st_ps = stps.tile([128, N], F32, tag="st")
                    for off, w in halves():
                        nc.tensor.matmul(st_ps[:, off:off + w],
                                         kt[:, j * 128:(j + 1) * 128],
                                         qt[:, off:off + w],
                                         start=True, stop=True)
                    pt = ptp.tile([128, N], F32R, tag="pt")
                    nc.scalar.activation(out=pt, in_=st_ps,
                                         func=mybir.ActivationFunctionType.Exp,
                                         scale=ikT[:, j, h].unsqueeze(-1))
                    for off, w in halves():
                        nc.tensor.matmul(av[0:E, off:off + w], va,
                                         pt[:, off:off + w],
                                         start=(j == 0), stop=(j == NT - 1))

                ao = aop.tile([E, N], F32R, tag="ao")
                nc.vector.tensor_copy(ao, av[0:E, :])
                nc.sync.dma_start(out=attn_d[b, h * 64:(h + 1) * 64, :], in_=ao[0:64])
                nc.sync.dma_start(out=den_all[h:h + 1, :],
                                  in_=ao[64:65, :].bitcast(F32))
            del va

            invden = statp.tile([H, N], F32R, tag="invden")
            for off, w in halves():
                dtmp = statp.tile([H, TCH], F32, tag="inv")
                nc.vector.reciprocal_approx_fast(out=dtmp[:, :w],
                                                 in_=den_all[:, off:off + w])
                nc.vector.tensor_copy(invden[:, off:off + w], dtmp[:, :w])

            # ================= projection (per token-half) ==================
            for off, w in halves():
                sl = slice(off, off + w)
                atn = []
                for k in range(KT):
                    raw = stagep.tile([128, TCH], F32R, tag="rawa")
                    nc.sync.dma_start(out=raw[:, :w],
                                      in_=attn_d[b, k * 128:(k + 1) * 128, sl])
                    at = atnp.tile([128, TCH], F32R, tag="atn")
                    bd = mmps.tile([128, TCH], F32, tag="mm")
                    nc.tensor.matmul(bd[:, :w], densel_sb[:, k * 128:(k + 1) * 128],
                                     invden[:, sl], start=True, stop=True)
                    nc.vector.tensor_mul(at[:, :w], raw[:, :w], bd[:, :w])
                    atn.append(at)

                for mt in range(KT):
                    pw = pwp.tile([128, KT, 128], F32R, tag="pw")
                    nc.gpsimd.dma_start(
                        out=pw, in_=proj_wT[mt].rearrange("p (k f) -> p k f", f=128))
                    ps = mmps.tile([128, TCH], F32, tag="mm")
                    for k in range(KT):
                        nc.tensor.matmul(ps[:, :w], pw[:, k], atn[k][:, :w],
                                         start=(k == 0), stop=(k == KT - 1))
                    yst = ystp.tile([128, TCH], F32, tag="yst")
                    nc.vector.tensor_scalar_add(yst[:, :w], ps[:, :w],
                                                bias_sb[:, mt:mt + 1])
                    nc.sync.dma_start(
                        out=yT[b, mt * 128:(mt + 1) * 128, sl],
                        in_=yst[:, :w])
                del atn

    return nc


def prep_inputs(x, qkv_w, proj_w, proj_b, q_norm_w, k_norm_w, n_cores):
    """Host-side prep: shard over batch, pre-transpose, build selector masks.
    Returns (in_maps, meta) where in_maps[i] is the input dict for core i."""
    B, N, C = x.shape
    H = C // 64
    Dh = 64
    B_local = B // n_cores
    scale = Dh ** -0.5

    qkv_wT = np.ascontiguousarray(qkv_w.T)          # [C, 3C]
    # per-Mtile contiguous layout [ft, p, k*128+f] so weight DMAs are linear
    qk_wT = np.ascontiguousarray(
        qkv_wT[:, :2 * C].reshape(C // 128, 128, 2 * C // 128, 128)
        .transpose(2, 1, 0, 3).reshape(2 * C // 128, 128, C))
    v_wT = np.ascontiguousarray(qkv_wT[:, 2 * C:])
    proj_wT = np.ascontiguousarray(
        proj_w.T.reshape(C // 128, 128, C // 128, 128)
        .transpose(2, 1, 0, 3).reshape(C // 128, 128, C))

    heads = np.arange(C) // Dh                       # head index per channel
    mask_ssq = (heads[:, None] == np.arange(H)[None, :]).astype(np.float32)
    w_qk = (q_norm_w * k_norm_w).astype(np.float32)  # [Dh]
    w_sel = np.zeros((H, C), np.float32)
    sel_q = np.zeros((H, C), np.float32)
    den_sel = np.zeros((H, C), np.float32)
    for h in range(H):
        w_sel[h, h * Dh:(h + 1) * Dh] = w_qk
        sel_q[h, h * Dh:(h + 1) * Dh] = scale * w_qk
        den_sel[h, h * Dh:(h + 1) * Dh] = 1.0

    shared = dict(qk_wT=qk_wT, v_wT=v_wT, proj_wT=proj_wT,
                  proj_b=proj_b.astype(np.float32), mask_ssq=mask_ssq,
                  w_sel=w_sel, sel_q=sel_q, den_sel=den_sel,
                  vinit=np.ones((128, H), np.float32))
    in_maps = []
    for i in range(n_cores):
        xs = x[i * B_local:(i + 1) * B_local]        # [B_local, N, C]
        xTl = np.ascontiguousarray(xs.transpose(0, 2, 1))  # [B_local, C, N]
        in_maps.append(dict(xT=xTl, **shared))
    return in_maps, dict(B=B, N=N, C=C, H=H, B_local=B_local)


def gather_output(results, meta):
    B, N, C, B_local = meta["B"], meta["N"], meta["C"], meta["B_local"]
    y = np.empty((B, N, C), np.float32)
    for i, r in enumerate(results):
        yTl = r["yT"]                                # [B_local, C, N]
        y[i * B_local:(i + 1) * B_local] = yTl.transpose(0, 2, 1)
    return y


N_CORES = 8
_CACHE = {}


def _get_nc():
    if "nc" not in _CACHE:
        from concourse import bacc

        nc = bacc.Bacc("TRN2", target_bir_lowering=False, debug=False,
                       num_devices=N_CORES)
        build_attention(nc, B_local=16 // N_CORES, N=1024, C=1024, H=16)
        nc.compile()
        _CACHE["nc"] = nc
    return _CACHE["nc"]


def run_sharded(in_maps, trace=False):
    from concourse.bass_utils import run_bass_kernel_spmd

    return run_bass_kernel_spmd(_get_nc(), in_maps,
                                core_ids=list(range(N_CORES)), trace=trace)


def kernel(x, qkv_w, proj_w, proj_b, q_norm_w, k_norm_w):
    x = np.asarray(x)
    in_maps, meta = prep_inputs(np.asarray(x), np.asarray(qkv_w),
                                np.asarray(proj_w), np.asarray(proj_b),
                                np.asarray(q_norm_w), np.asarray(k_norm_w),
                                N_CORES)
    res = run_sharded(in_maps)
    return gather_output(res.results, meta)

